# revision 12
# baseline (speedup 1.0000x reference)
"""BitLinear (1.58-bit) kernel for Trainium2, 8-core data-parallel SPMD.

Reference op: out = sign(x) @ ternarize(W).T where
  ternarize(W) = sign(W) * min(round(|W| / gamma), 1), gamma = mean(|W|) + 1e-6.

Strategy (per sharding hint: data-parallel over batch*seq, replicate ternary W):
  - Host: ternarize W once, transpose to [in, out], pack as fp8e4 bytes
    (exact for -1/0/+1).  Shard x by rows across 8 cores; send only the SIGN
    BITS of each x shard (8 contraction-slots per byte) - 0.5 MB per core
    instead of 4.2 MB.
  - Device (per core): expand sign bits to fp8 {+1,-1} bytes on DVE
    (shift/and then or-in the fp8 exponent bits), then dense fp8 DoubleRow
    matmuls (2 MACs/cell/cyc - the trn2 PE ceiling) accumulating in PSUM f32.
    Products are +-1 and row sums <= 2048 so f32 accumulation and the f16
    output are exact.
  - Host: concatenate + re-tile the 8 per-core outputs.

v2 schedule notes (measured-driven):
  - The PE stream is the hard floor: 512 DoubleRow matmuls x 216 ns = 110.6 us
    per core.  Everything else is startup/tail engineering.
  - All input DMAs use per-partition-contiguous multi-KB descriptor elements
    (the v1 strided groups moved 512 B/descriptor and crawled at ~140 GB/s).
  - Weights go q-major [128, QT, KC, 2, 512] and stream on the SYNC HWDGE
    ring in deadline order (q0 kc0, kc1, kc2-3, ... then q1, q2, q3); x sign
    bits + output stores ride the SCALAR HWDGE ring.  HWDGE rings drain
    FIFO per ring, so ring order IS the priority - no gate hacks needed.
  - The first mi-block of q0 runs kc-MAJOR (kc sweeps mi0-7 per chunk): the
    first matmul needs only 128 KB of weights + one x group instead of the
    whole 1 MB quarter, so the stream starts ~3 us into the measured window.
  - ~20 scratch warmup matmuls keep the PE busy from ~0.7 us so the HAM
    clock-gate reaches 2.4 GHz right as real work starts.
  - PSUM drains alternate ACT/DVE copies and sync/scalar store rings; the
    final unit splits into column halves so its copy+store overlaps the
    closing matmuls.

Layout: contraction index i in [0, 2048) is split as i = kc*256 + j*128 + p
(kc = 256-wide chunk, j = DoubleRow pair slot, p = SBUF partition).  Both
matmul operands are stored [128, ..., 2, N] in SBUF and sliced to the 3D
[128 part, 2, N] APs that MatmulPerfMode.DoubleRow requires.
"""

import numpy as np
import ml_dtypes

import concourse.bass as bass
import concourse.bacc as bacc
import concourse.mybir as mybir
from concourse.tile import TileContext
from concourse.bass_utils import run_bass_kernel_spmd

FP8 = ml_dtypes.float8_e4m3  # maps to mybir.dt.float8e4

N_CORES = 8
EPS = 1e-6

# Full-problem shapes (hardcoded per harness contract).
B, S, I_DIM, O_DIM = 4, 4096, 2048, 2048
M_TOT = B * S                 # 16384 rows
M_PER = M_TOT // N_CORES      # 2048 rows per core

KC = I_DIM // 256             # 8 contraction chunks
MT = M_PER // 128             # 16 output row tiles
QT = O_DIM // 512             # 4 output col quarters (one PSUM bank each)
MI_BLK = 8                    # m-tiles per schedule block

# DMA groups.  The x sign bits ride the scalar HWDGE ring (otherwise idle
# until output stores begin ~+15 us) while the weights stream on the sync
# ring, so the first x group and the first weight chunk land in parallel.
# Ring sharing is packet-round-robin, and until the x bits are done the
# weight ring is still on its 1-2 KB-packet head chunks, so neither side
# starves the other; the 8 KB bulk quarters queue strictly behind.
X_GROUPS = [(0, 4), (4, 8), (8, MT)]
W_GROUPS = [(0, 0, 1), (0, 1, 2), (0, 2, 4), (0, 4, 6), (0, 6, 8),
            (1, 0, 8), (2, 0, 8), (3, 0, 8)]            # (q, k0, k1)

# Sign-bit expansion order (m0, m1, kc): the head feeds the kc-major first
# block just ahead of consumption; (8, 16, kc) pairs are emitted later,
# interleaved into the q2 phase (deadline is the mi8-15 block at ~+58 us).
E_HEAD = [(0, 4, 0), (4, 8, 0), (0, 4, 1), (4, 8, 1),
          (0, 8, 2), (0, 8, 3), (0, 8, 4), (0, 8, 5), (0, 8, 6), (0, 8, 7)]
E_TAIL = [(8, 16, kc) for kc in range(KC)]

WU_N = 22                     # warmup matmuls (128-col, ~107 ns cold each)


def build_program() -> bass.Bass:
    """Per-core SPMD program: out[m, o] = sign(x)[m, :] @ Wq[o, :].T.

    DRAM inputs (flat u8, concatenated per-DMA-group partition-major blocks):
      xp : sign bits of x^T, byte [p, mi, j, m] holds bits kc=0..7
           (bit kc = 1 iff x < 0), i = kc*256 + j*128 + p
      wt : ternary Wq^T as fp8e4 bytes, q-major blocks [128p, kcr, 2, 512]
    DRAM output:
      out: [MT*QT*128, 512] f16; block (mi*QT + q) holds rows mi*128..+128,
           cols q*512..+512 (host re-tiles; integer values <= 2048, exact)
    """
    nc = bacc.Bacc()

    xp_total = 128 * MT * 2 * 128
    w_total = KC * 128 * 2 * O_DIM
    xp = nc.declare_dram_parameter(
        "xp", [xp_total], mybir.dt.uint8, isOutput=False)
    wt = nc.declare_dram_parameter(
        "wt", [w_total], mybir.dt.uint8, isOutput=False)
    out = nc.declare_dram_parameter(
        "out", [MT * QT * 128, 512], mybir.dt.float16, isOutput=True)

    with TileContext(nc) as tc:
        with (
            tc.tile_pool(name="wq", bufs=1) as wq_pool,
            tc.tile_pool(name="xs", bufs=1) as xs_pool,
            tc.tile_pool(name="xpk", bufs=1) as xp_pool,
            tc.tile_pool(name="psum", bufs=8, space="PSUM") as psum_pool,
            tc.tile_pool(name="osb", bufs=8) as out_pool,
        ):
            xp_sb = xp_pool.tile([128, MT, 2, 128], mybir.dt.uint8)
            xs_sb = xs_pool.tile([128, MT, KC, 2, 128], mybir.dt.float8e4)
            wq_sb = wq_pool.tile([128, QT, KC, 2, 512], mybir.dt.float8e4)

            x_off = {}
            off = 0
            for b0, b1 in X_GROUPS:
                x_off[(b0, b1)] = off
                off += 128 * (b1 - b0) * 2 * 128
            w_off = {}
            off = 0
            for g in W_GROUPS:
                w_off[g] = off
                off += 128 * (g[2] - g[1]) * 2 * 512

            # Warmup scratch: memsets first on the DVE queue (its real work
            # waits on x DMA anyway), so scratch matmuls start ~1 us into
            # the window and keep the PE busy until real operands land --
            # an idle PE gap here would reset the HAM warm-up window.
            wu_a = wq_pool.tile([128, 2, 128], mybir.dt.float8e4)
            wu_b = wq_pool.tile([128, 2, 128], mybir.dt.float8e4)
            nc.vector.memset(wu_a, 0.0)
            nc.vector.memset(wu_b, 0.0)

            def dma_w(g):
                q, k0, k1 = g
                sz = 128 * (k1 - k0) * 2 * 512
                o0 = w_off[g]
                nc.sync.dma_start(
                    out=wq_sb[:, q, k0:k1].bitcast(mybir.dt.uint8),
                    in_=wt[o0:o0 + sz].rearrange("(p r) -> p r", p=128))

            def dma_x(b0, b1):
                sz = 128 * (b1 - b0) * 2 * 128
                o0 = x_off[(b0, b1)]
                nc.scalar.dma_start(
                    out=xp_sb[:, b0:b1],
                    in_=xp[o0:o0 + sz].rearrange("(p r) -> p r", p=128))

            # All loads trigger up front; each ring drains FIFO in this
            # order, so the first-needed bytes own the wire and the bulk
            # quarters follow without flooding.
            for b0, b1 in X_GROUPS:
                dma_x(b0, b1)
            for g in W_GROUPS:
                dma_w(g)

            # PE warmup: scratch matmuls from ~+0.7 us keep the PE busy so
            # the HAM clock-gate fires (~+4.1 us) right as the real stream
            # gets going; sized to end as the first real operands land.
            wu_ps = psum_pool.tile([128, 128], mybir.dt.float32,
                                   name="wu_ps", tag="ps")
            for _ in range(WU_N):
                nc.tensor.matmul(wu_ps, wu_a, wu_b, start=True, stop=True,
                                 perf_mode=mybir.MatmulPerfMode.DoubleRow)

            xs_u32 = xs_sb.bitcast(mybir.dt.uint32)
            xp_u32 = xp_sb.bitcast(mybir.dt.uint32)

            def expand_x(m0, m1, kc):
                # Sign bits -> fp8 {+1,-1}: bit kc shifted to each byte's MSB
                # (fp8 sign bit), then OR in 0x38 (the fp8e4 encoding of 1.0).
                nc.vector.tensor_scalar(
                    out=xs_u32[:, m0:m1, kc], in0=xp_u32[:, m0:m1],
                    scalar1=7 - kc, scalar2=0x80808080,
                    op0=mybir.AluOpType.logical_shift_left,
                    op1=mybir.AluOpType.bitwise_and)
                nc.vector.tensor_scalar(
                    out=xs_u32[:, m0:m1, kc], in0=xs_u32[:, m0:m1, kc],
                    scalar1=0x38383838, scalar2=None,
                    op0=mybir.AluOpType.bitwise_or)

            for m0, m1, kc in E_HEAD:
                expand_x(m0, m1, kc)

            # PSUM drain: copy f32 -> f16 (exact) and store one contiguous
            # 128 KB block.  Copies alternate ACT/DVE and stores alternate
            # the sync/scalar rings so no single engine paces the drains.
            drain_ctr = [0]

            def drain(mi, q, ps, cols=None):
                blk = mi * QT + q
                k = drain_ctr[0]
                drain_ctr[0] += 1
                if cols is None:
                    ot = out_pool.tile([128, 512], mybir.dt.float16,
                                       name="ot", tag="ot")
                    dst = out[bass.ts(blk, 128)]
                else:
                    c0, c1 = cols
                    ot = out_pool.tile([128, c1 - c0], mybir.dt.float16,
                                       name="ot", tag="ot")
                    dst = out[bass.ts(blk, 128), c0:c1]
                if k % 2 == 0:
                    nc.scalar.copy(ot, ps)
                    nc.sync.dma_start(out=dst, in_=ot)
                else:
                    nc.vector.tensor_copy(ot, ps)
                    nc.scalar.dma_start(out=dst, in_=ot)

            def unit(mi, q):
                ps = psum_pool.tile([128, 512], mybir.dt.float32,
                                    name="ps", tag="ps")
                for kc in range(KC):
                    nc.tensor.matmul(
                        ps, xs_sb[:, mi, kc],
                        wq_sb[:, q, kc],
                        start=(kc == 0), stop=(kc == KC - 1),
                        perf_mode=mybir.MatmulPerfMode.DoubleRow)
                drain(mi, q, ps)

            def unit_split(mi, q):
                # Final unit: two independent column-half accumulations so
                # the first half's copy+store overlaps the second half's
                # matmuls, shrinking the end-of-kernel drain chain.
                for h in range(2):
                    ph = psum_pool.tile([128, 256], mybir.dt.float32,
                                        name="ps", tag="ps")
                    o0 = h * 256
                    for kc in range(KC):
                        nc.tensor.matmul(
                            ph, xs_sb[:, mi, kc],
                            wq_sb[:, q, kc, :, o0:o0 + 256],
                            start=(kc == 0), stop=(kc == KC - 1),
                            perf_mode=mybir.MatmulPerfMode.DoubleRow)
                    drain(mi, q, ph, cols=(o0, o0 + 256))

            # Block 0, quarter 0 runs kc-major: chunk kc sweeps mi0-7, so
            # matmuls start once (q0, kc0) and the first x group land; each
            # bank's last chunk is followed by its drain (staggered, one per
            # 8 matmuls).
            banks = [psum_pool.tile([128, 512], mybir.dt.float32,
                                    name=f"b{mi}", tag="ps")
                     for mi in range(MI_BLK)]
            for kc in range(KC):
                for mi in range(MI_BLK):
                    nc.tensor.matmul(
                        banks[mi], xs_sb[:, mi, kc], wq_sb[:, 0, kc],
                        start=(kc == 0), stop=(kc == KC - 1),
                        perf_mode=mybir.MatmulPerfMode.DoubleRow)
                    if kc == KC - 1:
                        drain(mi, 0, banks[mi])

            # Block 0, quarters 1-3 (weights long since resident): standard
            # per-unit accumulation.  The mi8-15 sign-bit expansion is
            # interleaved into the q2 phase (DVE is idle by then; the mi8-15
            # block starts ~+58 us).
            tail_iter = iter(E_TAIL)
            for q in range(1, QT):
                for mi in range(MI_BLK):
                    unit(mi, q)
                    if q == 2:
                        g = next(tail_iter, None)
                        if g is not None:
                            expand_x(*g)

            # Block 1: mi8-15, all quarters; last unit split for the tail.
            for q in range(QT):
                for mi in range(MI_BLK, MT):
                    if q == QT - 1 and mi == MT - 1:
                        unit_split(mi, q)
                    else:
                        unit(mi, q)

    nc.finalize()
    return nc


def ternarize_host(weight: np.ndarray) -> np.ndarray:
    """absmean ternarization, f64 for a faithful gamma; returns {-1,0,1} f32."""
    w = weight.astype(np.float64)
    gamma = np.mean(np.abs(w)) + EPS
    return (np.sign(w) * np.minimum(np.round(np.abs(w) / gamma), 1.0)).astype(
        np.float32)


def pack_w_flat(wq_t: np.ndarray) -> np.ndarray:
    """ternary Wq^T [i, o] f32 -> flat u8 (fp8e4 bytes), DMA-grouped."""
    # [kc, j, p, o] -> fp8 bytes
    w4 = wq_t.reshape(KC, 2, 128, O_DIM).astype(FP8).view(np.uint8)
    blocks = []
    for q, k0, k1 in W_GROUPS:
        blk = w4[k0:k1, :, :, q * 512:(q + 1) * 512]     # [kcr, 2, 128, 512]
        blocks.append(np.ascontiguousarray(
            blk.transpose(2, 0, 1, 3)).reshape(-1))      # partition-major
    return np.concatenate(blocks)


def pack_x_flat(sh: np.ndarray) -> np.ndarray:
    """x shard [m_per, i] f32 -> flat u8 sign-bit planes, DMA-grouped.

    Byte (p, mi, j, m) holds bit kc = signbit(x[mi*128+m, kc*256+j*128+p]).
    """
    sb = np.signbit(sh)                                   # [m, i] bool
    # [kc, j, p, mi, m] -> [p, mi, j, m, kc]
    b = sb.T.reshape(KC, 2, 128, MT, 128).transpose(2, 3, 1, 4, 0)
    pk = np.packbits(np.ascontiguousarray(b), axis=-1,
                     bitorder="little")[..., 0]           # [128, MT, 2, 128]
    blocks = [np.ascontiguousarray(pk[:, b0:b1]).reshape(-1)
              for b0, b1 in X_GROUPS]
    return np.concatenate(blocks)


def prep_in_maps(x: np.ndarray, weight: np.ndarray) -> list[dict]:
    wq = ternarize_host(weight)                    # [o, i] ternary
    wt = pack_w_flat(np.ascontiguousarray(wq.T))
    xf = x.reshape(M_TOT, I_DIM)
    return [{"xp": pack_x_flat(xf[c * M_PER:(c + 1) * M_PER]), "wt": wt}
            for c in range(N_CORES)]


_PROGRAM_CACHE: dict = {}


def _get_program() -> bass.Bass:
    if "nc" not in _PROGRAM_CACHE:
        _PROGRAM_CACHE["nc"] = build_program()
    return _PROGRAM_CACHE["nc"]


def _gather(results: list[dict]) -> np.ndarray:
    # per-core out [MT*QT*128, 512] -> [m_per, o]
    shards = [
        np.asarray(r["out"]).reshape(MT, QT, 128, 512)
        .transpose(0, 2, 1, 3).reshape(M_PER, O_DIM)
        for r in results]
    full = np.concatenate(shards, axis=0)
    return np.ascontiguousarray(full.reshape(B, S, O_DIM).astype(np.float32))


def kernel(x: np.ndarray, weight: np.ndarray) -> np.ndarray:
    nc = _get_program()
    in_maps = prep_in_maps(np.asarray(x), np.asarray(weight))
    res = run_bass_kernel_spmd(nc, in_maps, core_ids=list(range(N_CORES)))
    return _gather(res.results)


def kernel_traced(x: np.ndarray, weight: np.ndarray, **trace_kw):
    """Like kernel() but returns (output, BassKernelResults) with a trace."""
    nc = _get_program()
    in_maps = prep_in_maps(np.asarray(x), np.asarray(weight))
    res = run_bass_kernel_spmd(
        nc, in_maps, core_ids=list(range(N_CORES)), trace=True, **trace_kw)
    return _gather(res.results), res


# revision 13
# speedup vs baseline: 1.1965x; 1.1965x over previous
"""BitLinear (1.58-bit) kernel for Trainium2, 8-core data-parallel SPMD.

Reference op: out = sign(x) @ ternarize(W).T where
  ternarize(W) = sign(W) * min(round(|W| / gamma), 1), gamma = mean(|W|) + 1e-6.

Strategy (per sharding hint: data-parallel over batch*seq, replicate ternary W):
  - Host: ternarize W once, transpose to [in, out], pack as fp8e4 bytes
    (exact for -1/0/+1).  Shard x by rows across 8 cores; send only the SIGN
    BITS of each x shard (8 contraction-slots per byte) - 0.5 MB per core
    instead of 4.2 MB.
  - Device (per core): expand sign bits to fp8 {+1,-1} bytes on DVE
    (shift/and then or-in the fp8 exponent bits), then dense fp8 DoubleRow
    matmuls (2 MACs/cell/cyc - the trn2 PE ceiling) accumulating in PSUM f32.
    Products are +-1 and row sums <= 2048 so f32 accumulation and the f16
    output are exact.
  - Host: concatenate + re-tile the 8 per-core outputs.

v2 schedule notes (measured-driven):
  - The PE stream is the hard floor: 512 DoubleRow matmuls x 216 ns = 110.6 us
    per core.  Everything else is startup/tail engineering.
  - All input DMAs use per-partition-contiguous multi-KB descriptor elements
    (the v1 strided groups moved 512 B/descriptor and crawled at ~140 GB/s).
  - Weights go q-major [128, QT, KC, 2, 512] and stream on the SYNC HWDGE
    ring in deadline order (q0 kc0, kc1, kc2-3, ... then q1, q2, q3); x sign
    bits + output stores ride the SCALAR HWDGE ring.  HWDGE rings drain
    FIFO per ring, so ring order IS the priority - no gate hacks needed.
  - The first mi-block of q0 runs kc-MAJOR (kc sweeps mi0-7 per chunk): the
    first matmul needs only 128 KB of weights + one x group instead of the
    whole 1 MB quarter, so the stream starts ~3 us into the measured window.
  - ~20 scratch warmup matmuls keep the PE busy from ~0.7 us so the HAM
    clock-gate reaches 2.4 GHz right as real work starts.
  - PSUM drains alternate ACT/DVE copies and sync/scalar store rings; the
    final unit splits into column halves so its copy+store overlaps the
    closing matmuls.

Layout: contraction index i in [0, 2048) is split as i = kc*256 + j*128 + p
(kc = 256-wide chunk, j = DoubleRow pair slot, p = SBUF partition).  Both
matmul operands are stored [128, ..., 2, N] in SBUF and sliced to the 3D
[128 part, 2, N] APs that MatmulPerfMode.DoubleRow requires.
"""

import numpy as np
import ml_dtypes

import concourse.bass as bass
import concourse.bacc as bacc
import concourse.mybir as mybir
from concourse.tile import TileContext
from concourse.bass_utils import run_bass_kernel_spmd

FP8 = ml_dtypes.float8_e4m3  # maps to mybir.dt.float8e4

N_CORES = 8
EPS = 1e-6

# Full-problem shapes (hardcoded per harness contract).
B, S, I_DIM, O_DIM = 4, 4096, 2048, 2048
M_TOT = B * S                 # 16384 rows
M_PER = M_TOT // N_CORES      # 2048 rows per core

KC = I_DIM // 256             # 8 contraction chunks
MT = M_PER // 128             # 16 output row tiles
QT = O_DIM // 512             # 4 output col quarters (one PSUM bank each)
MI_BLK = 8                    # m-tiles per schedule block

# DMA groups.  ALL loads ride the sync HWDGE ring in one deadline-ordered
# FIFO (x groups interleaved between weight chunks); the scalar ring carries
# only output stores.  Two active load rings would round-robin at packet
# granularity and the 8 KB weight packets starve the 1-2 KB x packets.
X_GROUPS = [(0, 4), (4, 8), (8, MT)]
W_GROUPS = [(0, 0, 1), (0, 1, 2), (0, 2, 4), (0, 4, 6), (0, 6, 8),
            (1, 0, 8), (2, 0, 8), (3, 0, 8)]            # (q, k0, k1)
# interleaved ring order: name, group
LOAD_ORDER = [("x", X_GROUPS[0]), ("w", W_GROUPS[0]), ("x", X_GROUPS[1]),
              ("x", X_GROUPS[2]), ("w", W_GROUPS[1]), ("w", W_GROUPS[2]),
              ("w", W_GROUPS[3]), ("w", W_GROUPS[4]), ("w", W_GROUPS[5]),
              ("w", W_GROUPS[6]), ("w", W_GROUPS[7])]

# Sign-bit expansion order (m0, m1, kc): the head feeds the kc-major first
# block just ahead of consumption; (8, 16, kc) pairs are emitted later,
# interleaved into the q2 phase (deadline is the mi8-15 block at ~+58 us).
E_HEAD = [(0, 4, 0), (4, 8, 0), (0, 4, 1), (4, 8, 1),
          (0, 8, 2), (0, 8, 3), (0, 8, 4), (0, 8, 5), (0, 8, 6), (0, 8, 7)]
E_TAIL = [(8, 16, kc) for kc in range(KC)]

WU_N = 26                     # warmup matmuls (128-col, ~107 ns cold each)


def build_program() -> bass.Bass:
    """Per-core SPMD program: out[m, o] = sign(x)[m, :] @ Wq[o, :].T.

    DRAM inputs (flat u8, concatenated per-DMA-group partition-major blocks):
      xp : sign bits of x^T, byte [p, mi, j, m] holds bits kc=0..7
           (bit kc = 1 iff x < 0), i = kc*256 + j*128 + p
      wt : ternary Wq^T as fp8e4 bytes, q-major blocks [128p, kcr, 2, 512]
    DRAM output:
      out: [MT*QT*128, 512] f16; block (mi*QT + q) holds rows mi*128..+128,
           cols q*512..+512 (host re-tiles; integer values <= 2048, exact)
    """
    nc = bacc.Bacc()

    xp_total = 128 * MT * 2 * 128
    w_total = KC * 128 * 2 * O_DIM
    xp = nc.declare_dram_parameter(
        "xp", [xp_total], mybir.dt.uint8, isOutput=False)
    wt = nc.declare_dram_parameter(
        "wt", [w_total], mybir.dt.uint8, isOutput=False)
    out = nc.declare_dram_parameter(
        "out", [MT * QT * 128, 512], mybir.dt.float16, isOutput=True)

    with TileContext(nc) as tc:
        with (
            tc.tile_pool(name="wq", bufs=1) as wq_pool,
            tc.tile_pool(name="xs", bufs=1) as xs_pool,
            tc.tile_pool(name="xpk", bufs=1) as xp_pool,
            tc.tile_pool(name="psum", bufs=8, space="PSUM") as psum_pool,
            tc.tile_pool(name="osb", bufs=8) as out_pool,
        ):
            xp_sb = xp_pool.tile([128, MT, 2, 128], mybir.dt.uint8)
            xs_sb = xs_pool.tile([128, MT, KC, 2, 128], mybir.dt.float8e4)
            wq_sb = wq_pool.tile([128, QT, KC, 2, 512], mybir.dt.float8e4)

            x_off = {}
            off = 0
            for b0, b1 in X_GROUPS:
                x_off[(b0, b1)] = off
                off += 128 * (b1 - b0) * 2 * 128
            w_off = {}
            off = 0
            for g in W_GROUPS:
                w_off[g] = off
                off += 128 * (g[2] - g[1]) * 2 * 512

            # Warmup scratch: memsets first on the DVE queue (its real work
            # waits on x DMA anyway), so scratch matmuls start ~1 us into
            # the window and keep the PE busy until real operands land --
            # an idle PE gap here would reset the HAM warm-up window.
            wu_a = wq_pool.tile([128, 2, 128], mybir.dt.float8e4)
            wu_b = wq_pool.tile([128, 2, 128], mybir.dt.float8e4)
            nc.vector.memset(wu_a, 0.0)
            nc.vector.memset(wu_b, 0.0)

            def dma_w(g):
                q, k0, k1 = g
                sz = 128 * (k1 - k0) * 2 * 512
                o0 = w_off[g]
                nc.sync.dma_start(
                    out=wq_sb[:, q, k0:k1].bitcast(mybir.dt.uint8),
                    in_=wt[o0:o0 + sz].rearrange("(p r) -> p r", p=128))

            def dma_x(b0, b1):
                sz = 128 * (b1 - b0) * 2 * 128
                o0 = x_off[(b0, b1)]
                nc.sync.dma_start(
                    out=xp_sb[:, b0:b1],
                    in_=xp[o0:o0 + sz].rearrange("(p r) -> p r", p=128))

            # All loads trigger up front on the sync ring; it drains them
            # FIFO in this order, so the first-needed bytes own the wire and
            # the bulk quarters follow without flooding.
            for kind, g in LOAD_ORDER:
                if kind == "x":
                    dma_x(*g)
                else:
                    dma_w(g)

            # PE warmup: scratch matmuls from ~+0.7 us keep the PE busy so
            # the HAM clock-gate fires (~+4.1 us) right as the real stream
            # gets going; sized to end as the first real operands land.
            wu_ps = psum_pool.tile([128, 128], mybir.dt.float32,
                                   name="wu_ps", tag="ps")
            for _ in range(WU_N):
                nc.tensor.matmul(wu_ps, wu_a, wu_b, start=True, stop=True,
                                 perf_mode=mybir.MatmulPerfMode.DoubleRow)

            xs_u32 = xs_sb.bitcast(mybir.dt.uint32)
            xp_u32 = xp_sb.bitcast(mybir.dt.uint32)

            def expand_x(m0, m1, kc):
                # Sign bits -> fp8 {+1,-1}: bit kc shifted to each byte's MSB
                # (fp8 sign bit), then OR in 0x38 (the fp8e4 encoding of 1.0).
                nc.vector.tensor_scalar(
                    out=xs_u32[:, m0:m1, kc], in0=xp_u32[:, m0:m1],
                    scalar1=7 - kc, scalar2=0x80808080,
                    op0=mybir.AluOpType.logical_shift_left,
                    op1=mybir.AluOpType.bitwise_and)
                nc.vector.tensor_scalar(
                    out=xs_u32[:, m0:m1, kc], in0=xs_u32[:, m0:m1, kc],
                    scalar1=0x38383838, scalar2=None,
                    op0=mybir.AluOpType.bitwise_or)

            for m0, m1, kc in E_HEAD:
                expand_x(m0, m1, kc)

            # PSUM drain: copy f32 -> f16 (exact) and store one contiguous
            # 128 KB block.  Copies alternate ACT/DVE and stores alternate
            # the sync/scalar rings so no single engine paces the drains.
            drain_ctr = [0]

            def drain(mi, q, ps, cols=None):
                blk = mi * QT + q
                k = drain_ctr[0]
                drain_ctr[0] += 1
                if cols is None:
                    ot = out_pool.tile([128, 512], mybir.dt.float16,
                                       name="ot", tag="ot")
                    dst = out[bass.ts(blk, 128)]
                else:
                    c0, c1 = cols
                    ot = out_pool.tile([128, c1 - c0], mybir.dt.float16,
                                       name="ot", tag="ot")
                    dst = out[bass.ts(blk, 128), c0:c1]
                if k % 2 == 0:
                    nc.scalar.copy(ot, ps)
                    nc.sync.dma_start(out=dst, in_=ot)
                else:
                    nc.vector.tensor_copy(ot, ps)
                    nc.scalar.dma_start(out=dst, in_=ot)

            def unit(mi, q):
                ps = psum_pool.tile([128, 512], mybir.dt.float32,
                                    name="ps", tag="ps")
                for kc in range(KC):
                    nc.tensor.matmul(
                        ps, xs_sb[:, mi, kc],
                        wq_sb[:, q, kc],
                        start=(kc == 0), stop=(kc == KC - 1),
                        perf_mode=mybir.MatmulPerfMode.DoubleRow)
                drain(mi, q, ps)

            def unit_split(mi, q):
                # Final unit: two independent column-half accumulations so
                # the first half's copy+store overlaps the second half's
                # matmuls, shrinking the end-of-kernel drain chain.
                for h in range(2):
                    ph = psum_pool.tile([128, 256], mybir.dt.float32,
                                        name="ps", tag="ps")
                    o0 = h * 256
                    for kc in range(KC):
                        nc.tensor.matmul(
                            ph, xs_sb[:, mi, kc],
                            wq_sb[:, q, kc, :, o0:o0 + 256],
                            start=(kc == 0), stop=(kc == KC - 1),
                            perf_mode=mybir.MatmulPerfMode.DoubleRow)
                    drain(mi, q, ph, cols=(o0, o0 + 256))

            # Block 0, quarter 0 runs kc-major: chunk kc sweeps mi0-7, so
            # matmuls start once (q0, kc0) and the first x group land; each
            # bank's last chunk is followed by its drain (staggered, one per
            # 8 matmuls).
            banks = [psum_pool.tile([128, 512], mybir.dt.float32,
                                    name=f"b{mi}", tag="ps")
                     for mi in range(MI_BLK)]
            for kc in range(KC):
                for mi in range(MI_BLK):
                    nc.tensor.matmul(
                        banks[mi], xs_sb[:, mi, kc], wq_sb[:, 0, kc],
                        start=(kc == 0), stop=(kc == KC - 1),
                        perf_mode=mybir.MatmulPerfMode.DoubleRow)
                    if kc == KC - 1:
                        drain(mi, 0, banks[mi])

            # Block 0, quarters 1-3 (weights long since resident): standard
            # per-unit accumulation.  The mi8-15 sign-bit expansion is
            # interleaved into the q2 phase (DVE is idle by then; the mi8-15
            # block starts ~+58 us).
            tail_iter = iter(E_TAIL)
            for q in range(1, QT):
                for mi in range(MI_BLK):
                    unit(mi, q)
                    if q == 2:
                        g = next(tail_iter, None)
                        if g is not None:
                            expand_x(*g)

            # Block 1: mi8-15, all quarters; last unit split for the tail.
            for q in range(QT):
                for mi in range(MI_BLK, MT):
                    if q == QT - 1 and mi == MT - 1:
                        unit_split(mi, q)
                    else:
                        unit(mi, q)

    nc.finalize()
    return nc


def ternarize_host(weight: np.ndarray) -> np.ndarray:
    """absmean ternarization, f64 for a faithful gamma; returns {-1,0,1} f32."""
    w = weight.astype(np.float64)
    gamma = np.mean(np.abs(w)) + EPS
    return (np.sign(w) * np.minimum(np.round(np.abs(w) / gamma), 1.0)).astype(
        np.float32)


def pack_w_flat(wq_t: np.ndarray) -> np.ndarray:
    """ternary Wq^T [i, o] f32 -> flat u8 (fp8e4 bytes), DMA-grouped."""
    # [kc, j, p, o] -> fp8 bytes
    w4 = wq_t.reshape(KC, 2, 128, O_DIM).astype(FP8).view(np.uint8)
    blocks = []
    for q, k0, k1 in W_GROUPS:
        blk = w4[k0:k1, :, :, q * 512:(q + 1) * 512]     # [kcr, 2, 128, 512]
        blocks.append(np.ascontiguousarray(
            blk.transpose(2, 0, 1, 3)).reshape(-1))      # partition-major
    return np.concatenate(blocks)


def pack_x_flat(sh: np.ndarray) -> np.ndarray:
    """x shard [m_per, i] f32 -> flat u8 sign-bit planes, DMA-grouped.

    Byte (p, mi, j, m) holds bit kc = signbit(x[mi*128+m, kc*256+j*128+p]).
    """
    sb = np.signbit(sh)                                   # [m, i] bool
    # [kc, j, p, mi, m] -> [p, mi, j, m, kc]
    b = sb.T.reshape(KC, 2, 128, MT, 128).transpose(2, 3, 1, 4, 0)
    pk = np.packbits(np.ascontiguousarray(b), axis=-1,
                     bitorder="little")[..., 0]           # [128, MT, 2, 128]
    blocks = [np.ascontiguousarray(pk[:, b0:b1]).reshape(-1)
              for b0, b1 in X_GROUPS]
    return np.concatenate(blocks)


def prep_in_maps(x: np.ndarray, weight: np.ndarray) -> list[dict]:
    wq = ternarize_host(weight)                    # [o, i] ternary
    wt = pack_w_flat(np.ascontiguousarray(wq.T))
    xf = x.reshape(M_TOT, I_DIM)
    return [{"xp": pack_x_flat(xf[c * M_PER:(c + 1) * M_PER]), "wt": wt}
            for c in range(N_CORES)]


_PROGRAM_CACHE: dict = {}


def _get_program() -> bass.Bass:
    if "nc" not in _PROGRAM_CACHE:
        _PROGRAM_CACHE["nc"] = build_program()
    return _PROGRAM_CACHE["nc"]


def _gather(results: list[dict]) -> np.ndarray:
    # per-core out [MT*QT*128, 512] -> [m_per, o]
    shards = [
        np.asarray(r["out"]).reshape(MT, QT, 128, 512)
        .transpose(0, 2, 1, 3).reshape(M_PER, O_DIM)
        for r in results]
    full = np.concatenate(shards, axis=0)
    return np.ascontiguousarray(full.reshape(B, S, O_DIM).astype(np.float32))


def kernel(x: np.ndarray, weight: np.ndarray) -> np.ndarray:
    nc = _get_program()
    in_maps = prep_in_maps(np.asarray(x), np.asarray(weight))
    res = run_bass_kernel_spmd(nc, in_maps, core_ids=list(range(N_CORES)))
    return _gather(res.results)


def kernel_traced(x: np.ndarray, weight: np.ndarray, **trace_kw):
    """Like kernel() but returns (output, BassKernelResults) with a trace."""
    nc = _get_program()
    in_maps = prep_in_maps(np.asarray(x), np.asarray(weight))
    res = run_bass_kernel_spmd(
        nc, in_maps, core_ids=list(range(N_CORES)), trace=True, **trace_kw)
    return _gather(res.results), res


# revision 15
# speedup vs baseline: 1.2042x; 1.0064x over previous
"""BitLinear (1.58-bit) kernel for Trainium2, 8-core data-parallel SPMD.

Reference op: out = sign(x) @ ternarize(W).T where
  ternarize(W) = sign(W) * min(round(|W| / gamma), 1), gamma = mean(|W|) + 1e-6.

Strategy (per sharding hint: data-parallel over batch*seq, replicate ternary W):
  - Host: ternarize W once, transpose to [in, out], pack as fp8e4 bytes
    (exact for -1/0/+1).  Shard x by rows across 8 cores; send only the SIGN
    BITS of each x shard (8 contraction-slots per byte) - 0.5 MB per core
    instead of 4.2 MB.
  - Device (per core): expand sign bits to fp8 {+1,-1} bytes on DVE
    (shift/and then or-in the fp8 exponent bits), then dense fp8 DoubleRow
    matmuls (2 MACs/cell/cyc - the trn2 PE ceiling) accumulating in PSUM f32.
    Products are +-1 and row sums <= 2048 so f32 accumulation and the f16
    output are exact.
  - Host: concatenate + re-tile the 8 per-core outputs.

v2 schedule notes (measured-driven):
  - The PE stream is the hard floor: 512 DoubleRow matmuls x 216 ns = 110.6 us
    per core.  Everything else is startup/tail engineering.
  - All input DMAs use per-partition-contiguous multi-KB descriptor elements
    (the v1 strided groups moved 512 B/descriptor and crawled at ~140 GB/s).
  - Weights go q-major [128, QT, KC, 2, 512] and stream on the SYNC HWDGE
    ring in deadline order (q0 kc0, kc1, kc2-3, ... then q1, q2, q3); x sign
    bits + output stores ride the SCALAR HWDGE ring.  HWDGE rings drain
    FIFO per ring, so ring order IS the priority - no gate hacks needed.
  - The first mi-block of q0 runs kc-MAJOR (kc sweeps mi0-7 per chunk): the
    first matmul needs only 128 KB of weights + one x group instead of the
    whole 1 MB quarter, so the stream starts ~3 us into the measured window.
  - ~20 scratch warmup matmuls keep the PE busy from ~0.7 us so the HAM
    clock-gate reaches 2.4 GHz right as real work starts.
  - PSUM drains alternate ACT/DVE copies and sync/scalar store rings; the
    final unit splits into column halves so its copy+store overlaps the
    closing matmuls.

Layout: contraction index i in [0, 2048) is split as i = kc*256 + j*128 + p
(kc = 256-wide chunk, j = DoubleRow pair slot, p = SBUF partition).  Both
matmul operands are stored [128, ..., 2, N] in SBUF and sliced to the 3D
[128 part, 2, N] APs that MatmulPerfMode.DoubleRow requires.
"""

import numpy as np
import ml_dtypes

import concourse.bass as bass
import concourse.bacc as bacc
import concourse.mybir as mybir
from concourse.tile import TileContext
from concourse.bass_utils import run_bass_kernel_spmd

FP8 = ml_dtypes.float8_e4m3  # maps to mybir.dt.float8e4

N_CORES = 8
EPS = 1e-6

# Full-problem shapes (hardcoded per harness contract).
B, S, I_DIM, O_DIM = 4, 4096, 2048, 2048
M_TOT = B * S                 # 16384 rows
M_PER = M_TOT // N_CORES      # 2048 rows per core

KC = I_DIM // 256             # 8 contraction chunks
MT = M_PER // 128             # 16 output row tiles
QT = O_DIM // 512             # 4 output col quarters (one PSUM bank each)
MI_BLK = 8                    # m-tiles per schedule block

# DMA groups.  ALL loads ride the sync HWDGE ring in one deadline-ordered
# FIFO (x groups interleaved between weight chunks); the scalar ring carries
# only output stores.  Two active load rings would round-robin at packet
# granularity and the 8 KB weight packets starve the 1-2 KB x packets.
X_GROUPS = [(0, 4), (4, 8), (8, MT)]
W_GROUPS = [(0, 0, 1), (0, 1, 2), (0, 2, 4), (0, 4, 6), (0, 6, 8),
            (1, 0, 8), (2, 0, 8), (3, 0, 8)]            # (q, k0, k1)
# interleaved ring order: name, group
LOAD_ORDER = [("x", X_GROUPS[0]), ("w", W_GROUPS[0]), ("x", X_GROUPS[1]),
              ("x", X_GROUPS[2]), ("w", W_GROUPS[1]), ("w", W_GROUPS[2]),
              ("w", W_GROUPS[3]), ("w", W_GROUPS[4]), ("w", W_GROUPS[5]),
              ("w", W_GROUPS[6]), ("w", W_GROUPS[7])]

# Sign-bit expansion order (m0, m1, kc): the head feeds the kc-major first
# block just ahead of consumption; (8, 16, kc) pairs are emitted later,
# interleaved into the q2 phase (deadline is the mi8-15 block at ~+58 us).
E_HEAD = [(0, 2, 0), (2, 4, 0), (4, 8, 0), (0, 4, 1), (4, 8, 1),
          (0, 8, 2), (0, 8, 3), (0, 8, 4), (0, 8, 5), (0, 8, 6), (0, 8, 7)]
E_TAIL = [(8, 16, kc) for kc in range(KC)]

WU_N = 26                     # warmup matmuls (128-col, ~107 ns cold each)


def build_program() -> bass.Bass:
    """Per-core SPMD program: out[m, o] = sign(x)[m, :] @ Wq[o, :].T.

    DRAM inputs (flat u8, concatenated per-DMA-group partition-major blocks):
      xp : sign bits of x^T, byte [p, mi, j, m] holds bits kc=0..7
           (bit kc = 1 iff x < 0), i = kc*256 + j*128 + p
      wt : ternary Wq^T as fp8e4 bytes, q-major blocks [128p, kcr, 2, 512]
    DRAM output:
      out: [MT*QT*128, 512] f16; block (mi*QT + q) holds rows mi*128..+128,
           cols q*512..+512 (host re-tiles; integer values <= 2048, exact)
    """
    nc = bacc.Bacc()

    xp_total = 128 * MT * 2 * 128
    w_total = KC * 128 * 2 * O_DIM
    xp = nc.declare_dram_parameter(
        "xp", [xp_total], mybir.dt.uint8, isOutput=False)
    wt = nc.declare_dram_parameter(
        "wt", [w_total], mybir.dt.uint8, isOutput=False)
    out = nc.declare_dram_parameter(
        "out", [MT * QT * 128, 512], mybir.dt.float16, isOutput=True)

    with TileContext(nc) as tc:
        with (
            tc.tile_pool(name="wq", bufs=1) as wq_pool,
            tc.tile_pool(name="xs", bufs=1) as xs_pool,
            tc.tile_pool(name="xpk", bufs=1) as xp_pool,
            tc.tile_pool(name="psum", bufs=8, space="PSUM") as psum_pool,
            tc.tile_pool(name="osb", bufs=8) as out_pool,
        ):
            xp_sb = xp_pool.tile([128, MT, 2, 128], mybir.dt.uint8)
            xs_sb = xs_pool.tile([128, MT, KC, 2, 128], mybir.dt.float8e4)
            wq_sb = wq_pool.tile([128, QT, KC, 2, 512], mybir.dt.float8e4)

            x_off = {}
            off = 0
            for b0, b1 in X_GROUPS:
                x_off[(b0, b1)] = off
                off += 128 * (b1 - b0) * 2 * 128
            w_off = {}
            off = 0
            for g in W_GROUPS:
                w_off[g] = off
                off += 128 * (g[2] - g[1]) * 2 * 512

            # Warmup scratch: memsets first on the DVE queue (its real work
            # waits on x DMA anyway), so scratch matmuls start ~1 us into
            # the window and keep the PE busy until real operands land --
            # an idle PE gap here would reset the HAM warm-up window.
            wu_a = wq_pool.tile([128, 2, 128], mybir.dt.float8e4)
            wu_b = wq_pool.tile([128, 2, 128], mybir.dt.float8e4)
            nc.vector.memset(wu_a, 0.0)
            nc.vector.memset(wu_b, 0.0)

            def dma_w(g):
                q, k0, k1 = g
                sz = 128 * (k1 - k0) * 2 * 512
                o0 = w_off[g]
                nc.sync.dma_start(
                    out=wq_sb[:, q, k0:k1].bitcast(mybir.dt.uint8),
                    in_=wt[o0:o0 + sz].rearrange("(p r) -> p r", p=128))

            def dma_x(b0, b1):
                sz = 128 * (b1 - b0) * 2 * 128
                o0 = x_off[(b0, b1)]
                nc.sync.dma_start(
                    out=xp_sb[:, b0:b1],
                    in_=xp[o0:o0 + sz].rearrange("(p r) -> p r", p=128))

            # All loads trigger up front on the sync ring; it drains them
            # FIFO in this order, so the first-needed bytes own the wire and
            # the bulk quarters follow without flooding.
            for kind, g in LOAD_ORDER:
                if kind == "x":
                    dma_x(*g)
                else:
                    dma_w(g)

            # PE warmup: scratch matmuls from ~+0.7 us keep the PE busy so
            # the HAM clock-gate fires (~+4.1 us) right as the real stream
            # gets going; sized to end as the first real operands land.
            wu_ps = psum_pool.tile([128, 128], mybir.dt.float32,
                                   name="wu_ps", tag="ps")
            for _ in range(WU_N):
                nc.tensor.matmul(wu_ps, wu_a, wu_b, start=True, stop=True,
                                 perf_mode=mybir.MatmulPerfMode.DoubleRow)

            xs_u32 = xs_sb.bitcast(mybir.dt.uint32)
            xp_u32 = xp_sb.bitcast(mybir.dt.uint32)

            def expand_x(m0, m1, kc):
                # Sign bits -> fp8 {+1,-1}: bit kc shifted to each byte's MSB
                # (fp8 sign bit), then OR in 0x38 (the fp8e4 encoding of 1.0).
                nc.vector.tensor_scalar(
                    out=xs_u32[:, m0:m1, kc], in0=xp_u32[:, m0:m1],
                    scalar1=7 - kc, scalar2=0x80808080,
                    op0=mybir.AluOpType.logical_shift_left,
                    op1=mybir.AluOpType.bitwise_and)
                nc.vector.tensor_scalar(
                    out=xs_u32[:, m0:m1, kc], in0=xs_u32[:, m0:m1, kc],
                    scalar1=0x38383838, scalar2=None,
                    op0=mybir.AluOpType.bitwise_or)

            for m0, m1, kc in E_HEAD:
                expand_x(m0, m1, kc)

            # PSUM drain: copy f32 -> f16 (exact) and store one contiguous
            # 128 KB block.  Copies alternate ACT/DVE and stores alternate
            # the sync/scalar rings so no single engine paces the drains.
            drain_ctr = [0]

            def drain(mi, q, ps, cols=None):
                blk = mi * QT + q
                k = drain_ctr[0]
                drain_ctr[0] += 1
                if cols is None:
                    ot = out_pool.tile([128, 512], mybir.dt.float16,
                                       name="ot", tag="ot")
                    dst = out[bass.ts(blk, 128)]
                else:
                    c0, c1 = cols
                    ot = out_pool.tile([128, c1 - c0], mybir.dt.float16,
                                       name="ot", tag="ot")
                    dst = out[bass.ts(blk, 128), c0:c1]
                if k % 2 == 0:
                    nc.scalar.copy(ot, ps)
                    nc.sync.dma_start(out=dst, in_=ot)
                else:
                    nc.vector.tensor_copy(ot, ps)
                    nc.scalar.dma_start(out=dst, in_=ot)

            def unit(mi, q):
                ps = psum_pool.tile([128, 512], mybir.dt.float32,
                                    name="ps", tag="ps")
                for kc in range(KC):
                    nc.tensor.matmul(
                        ps, xs_sb[:, mi, kc],
                        wq_sb[:, q, kc],
                        start=(kc == 0), stop=(kc == KC - 1),
                        perf_mode=mybir.MatmulPerfMode.DoubleRow)
                drain(mi, q, ps)

            def unit_split(mi, q):
                # Final unit: four independent column-slice accumulations so
                # the earlier slices' copy+store overlap the later slices'
                # matmuls, shrinking the end-of-kernel drain chain to one
                # 128-col copy + 32 KB store + receipt.
                for h in range(4):
                    ph = psum_pool.tile([128, 128], mybir.dt.float32,
                                        name="ps", tag="ps")
                    o0 = h * 128
                    for kc in range(KC):
                        nc.tensor.matmul(
                            ph, xs_sb[:, mi, kc],
                            wq_sb[:, q, kc, :, o0:o0 + 128],
                            start=(kc == 0), stop=(kc == KC - 1),
                            perf_mode=mybir.MatmulPerfMode.DoubleRow)
                    drain(mi, q, ph, cols=(o0, o0 + 128))

            # Block 0, quarter 0 runs kc-major: chunk kc sweeps mi0-7, so
            # matmuls start once (q0, kc0) and the first x group land; each
            # bank's last chunk is followed by its drain (staggered, one per
            # 8 matmuls).
            banks = [psum_pool.tile([128, 512], mybir.dt.float32,
                                    name=f"b{mi}", tag="ps")
                     for mi in range(MI_BLK)]
            for kc in range(KC):
                for mi in range(MI_BLK):
                    nc.tensor.matmul(
                        banks[mi], xs_sb[:, mi, kc], wq_sb[:, 0, kc],
                        start=(kc == 0), stop=(kc == KC - 1),
                        perf_mode=mybir.MatmulPerfMode.DoubleRow)
                    if kc == KC - 1:
                        drain(mi, 0, banks[mi])

            # Block 0, quarters 1-3 (weights long since resident): standard
            # per-unit accumulation.  The mi8-15 sign-bit expansion is
            # interleaved into the q2 phase (DVE is idle by then; the mi8-15
            # block starts ~+58 us).
            tail_iter = iter(E_TAIL)
            for q in range(1, QT):
                for mi in range(MI_BLK):
                    unit(mi, q)
                    if q == 2:
                        g = next(tail_iter, None)
                        if g is not None:
                            expand_x(*g)

            # Block 1: mi8-15, all quarters; last unit split for the tail.
            for q in range(QT):
                for mi in range(MI_BLK, MT):
                    if q == QT - 1 and mi == MT - 1:
                        unit_split(mi, q)
                    else:
                        unit(mi, q)

    nc.finalize()
    return nc


def ternarize_host(weight: np.ndarray) -> np.ndarray:
    """absmean ternarization, f64 for a faithful gamma; returns {-1,0,1} f32."""
    w = weight.astype(np.float64)
    gamma = np.mean(np.abs(w)) + EPS
    return (np.sign(w) * np.minimum(np.round(np.abs(w) / gamma), 1.0)).astype(
        np.float32)


def pack_w_flat(wq_t: np.ndarray) -> np.ndarray:
    """ternary Wq^T [i, o] f32 -> flat u8 (fp8e4 bytes), DMA-grouped."""
    # [kc, j, p, o] -> fp8 bytes
    w4 = wq_t.reshape(KC, 2, 128, O_DIM).astype(FP8).view(np.uint8)
    blocks = []
    for q, k0, k1 in W_GROUPS:
        blk = w4[k0:k1, :, :, q * 512:(q + 1) * 512]     # [kcr, 2, 128, 512]
        blocks.append(np.ascontiguousarray(
            blk.transpose(2, 0, 1, 3)).reshape(-1))      # partition-major
    return np.concatenate(blocks)


def pack_x_flat(sh: np.ndarray) -> np.ndarray:
    """x shard [m_per, i] f32 -> flat u8 sign-bit planes, DMA-grouped.

    Byte (p, mi, j, m) holds bit kc = signbit(x[mi*128+m, kc*256+j*128+p]).
    """
    sb = np.signbit(sh)                                   # [m, i] bool
    # [kc, j, p, mi, m] -> [p, mi, j, m, kc]
    b = sb.T.reshape(KC, 2, 128, MT, 128).transpose(2, 3, 1, 4, 0)
    pk = np.packbits(np.ascontiguousarray(b), axis=-1,
                     bitorder="little")[..., 0]           # [128, MT, 2, 128]
    blocks = [np.ascontiguousarray(pk[:, b0:b1]).reshape(-1)
              for b0, b1 in X_GROUPS]
    return np.concatenate(blocks)


def prep_in_maps(x: np.ndarray, weight: np.ndarray) -> list[dict]:
    wq = ternarize_host(weight)                    # [o, i] ternary
    wt = pack_w_flat(np.ascontiguousarray(wq.T))
    xf = x.reshape(M_TOT, I_DIM)
    return [{"xp": pack_x_flat(xf[c * M_PER:(c + 1) * M_PER]), "wt": wt}
            for c in range(N_CORES)]


_PROGRAM_CACHE: dict = {}


def _get_program() -> bass.Bass:
    if "nc" not in _PROGRAM_CACHE:
        _PROGRAM_CACHE["nc"] = build_program()
    return _PROGRAM_CACHE["nc"]


def _gather(results: list[dict]) -> np.ndarray:
    # per-core out [MT*QT*128, 512] -> [m_per, o]
    shards = [
        np.asarray(r["out"]).reshape(MT, QT, 128, 512)
        .transpose(0, 2, 1, 3).reshape(M_PER, O_DIM)
        for r in results]
    full = np.concatenate(shards, axis=0)
    return np.ascontiguousarray(full.reshape(B, S, O_DIM).astype(np.float32))


def kernel(x: np.ndarray, weight: np.ndarray) -> np.ndarray:
    nc = _get_program()
    in_maps = prep_in_maps(np.asarray(x), np.asarray(weight))
    res = run_bass_kernel_spmd(nc, in_maps, core_ids=list(range(N_CORES)))
    return _gather(res.results)


def kernel_traced(x: np.ndarray, weight: np.ndarray, **trace_kw):
    """Like kernel() but returns (output, BassKernelResults) with a trace."""
    nc = _get_program()
    in_maps = prep_in_maps(np.asarray(x), np.asarray(weight))
    res = run_bass_kernel_spmd(
        nc, in_maps, core_ids=list(range(N_CORES)), trace=True, **trace_kw)
    return _gather(res.results), res


# revision 22
# speedup vs baseline: 1.2059x; 1.0014x over previous
"""BitLinear (1.58-bit) kernel for Trainium2, 8-core data-parallel SPMD.

Reference op: out = sign(x) @ ternarize(W).T where
  ternarize(W) = sign(W) * min(round(|W| / gamma), 1), gamma = mean(|W|) + 1e-6.

Strategy (per sharding hint: data-parallel over batch*seq, replicate ternary W):
  - Host: ternarize W once, transpose to [in, out], pack as fp8e4 bytes
    (exact for -1/0/+1).  Shard x by rows across 8 cores; send only the SIGN
    BITS of each x shard (8 contraction-slots per byte) - 0.5 MB per core
    instead of 4.2 MB.
  - Device (per core): expand sign bits to fp8 {+1,-1} bytes on DVE
    (shift/and then or-in the fp8 exponent bits), then dense fp8 DoubleRow
    matmuls (2 MACs/cell/cyc - the trn2 PE ceiling) accumulating in PSUM f32.
    Products are +-1 and row sums <= 2048 so f32 accumulation and the f16
    output are exact.
  - Host: concatenate + re-tile the 8 per-core outputs.

v2 schedule notes (measured-driven):
  - The PE stream is the hard floor: 512 DoubleRow matmuls x 216 ns = 110.6 us
    per core.  Everything else is startup/tail engineering.
  - All input DMAs use per-partition-contiguous multi-KB descriptor elements
    (the v1 strided groups moved 512 B/descriptor and crawled at ~140 GB/s).
  - Weights go q-major [128, QT, KC, 2, 512] and stream on the SYNC HWDGE
    ring in deadline order (q0 kc0, kc1, kc2-3, ... then q1, q2, q3); x sign
    bits + output stores ride the SCALAR HWDGE ring.  HWDGE rings drain
    FIFO per ring, so ring order IS the priority - no gate hacks needed.
  - The first mi-block of q0 runs kc-MAJOR (kc sweeps mi0-7 per chunk): the
    first matmul needs only 128 KB of weights + one x group instead of the
    whole 1 MB quarter, so the stream starts ~3 us into the measured window.
  - ~20 scratch warmup matmuls keep the PE busy from ~0.7 us so the HAM
    clock-gate reaches 2.4 GHz right as real work starts.
  - PSUM drains alternate ACT/DVE copies and sync/scalar store rings; the
    final unit splits into column halves so its copy+store overlaps the
    closing matmuls.

Layout: contraction index i in [0, 2048) is split as i = kc*256 + j*128 + p
(kc = 256-wide chunk, j = DoubleRow pair slot, p = SBUF partition).  Both
matmul operands are stored [128, ..., 2, N] in SBUF and sliced to the 3D
[128 part, 2, N] APs that MatmulPerfMode.DoubleRow requires.
"""

import numpy as np
import ml_dtypes

import concourse.bass as bass
import concourse.bacc as bacc
import concourse.mybir as mybir
from concourse.tile import TileContext
from concourse.bass_utils import run_bass_kernel_spmd

FP8 = ml_dtypes.float8_e4m3  # maps to mybir.dt.float8e4

N_CORES = 8
EPS = 1e-6

# Full-problem shapes (hardcoded per harness contract).
B, S, I_DIM, O_DIM = 4, 4096, 2048, 2048
M_TOT = B * S                 # 16384 rows
M_PER = M_TOT // N_CORES      # 2048 rows per core

KC = I_DIM // 256             # 8 contraction chunks
MT = M_PER // 128             # 16 output row tiles
QT = O_DIM // 512             # 4 output col quarters (one PSUM bank each)
MI_BLK = 8                    # m-tiles per schedule block

# DMA groups.  ALL loads ride the sync HWDGE ring in one deadline-ordered
# FIFO (x groups interleaved between weight chunks); the scalar ring carries
# only output stores.  Two active load rings would round-robin at packet
# granularity and the 8 KB weight packets starve the 1-2 KB x packets.
# The first two contraction chunks of mi0-7 additionally ship PRE-EXPANDED
# as fp8 bytes (xr, 2 x 256 KB) straight into the xs tile, so the first
# ~3.5 us of matmuls have no DVE expansion on their critical path; packed
# sign bits (xp) cover everything else, expanded at 1.73 us/chunk pace.
X_GROUPS = [(0, 8), (8, MT)]
XR_KC = 2                     # pre-expanded chunks (kc0, kc1) for mi0-7
W_GROUPS = [(0, 0, 1), (0, 1, 2), (0, 2, 4), (0, 4, 6), (0, 6, 8),
            (1, 0, 8), (2, 0, 8), (3, 0, 8)]            # (q, k0, k1)
# interleaved ring order: (kind, group)
LOAD_ORDER = [("r", 0), ("w", W_GROUPS[0]), ("r", 1), ("w", W_GROUPS[1]),
              ("x", X_GROUPS[0]), ("w", W_GROUPS[2]), ("x", X_GROUPS[1]),
              ("w", W_GROUPS[3]), ("w", W_GROUPS[4]), ("w", W_GROUPS[5]),
              ("w", W_GROUPS[6]), ("w", W_GROUPS[7])]

# Sign-bit expansion order (m0, m1, kc): mi0-7 kc0-1 arrive pre-expanded,
# so the DVE head starts at kc2; (8, 16, kc) pairs are emitted later,
# interleaved into the q2 phase (deadline is the mi8-15 block at ~+58 us).
E_HEAD = [(0, 8, kc) for kc in range(XR_KC, KC)]
E_TAIL = [(8, 16, kc) for kc in range(KC)]

WU_N = 24                     # warmup matmuls (128-col, ~107 ns cold each)


def build_program() -> bass.Bass:
    """Per-core SPMD program: out[m, o] = sign(x)[m, :] @ Wq[o, :].T.

    DRAM inputs (flat u8, concatenated per-DMA-group partition-major blocks):
      xp : sign bits of x^T, byte [p, mi, j, m] holds bits kc=0..7
           (bit kc = 1 iff x < 0), i = kc*256 + j*128 + p
      wt : ternary Wq^T as fp8e4 bytes, q-major blocks [128p, kcr, 2, 512]
    DRAM output:
      out: [MT*QT*128, 512] f16; block (mi*QT + q) holds rows mi*128..+128,
           cols q*512..+512 (host re-tiles; integer values <= 2048, exact)
    """
    nc = bacc.Bacc()

    xp_total = 128 * MT * 2 * 128
    xr_total = 128 * XR_KC * MI_BLK * 2 * 128
    w_total = KC * 128 * 2 * O_DIM
    xp = nc.declare_dram_parameter(
        "xp", [xp_total], mybir.dt.uint8, isOutput=False)
    xr = nc.declare_dram_parameter(
        "xr", [xr_total], mybir.dt.uint8, isOutput=False)
    wt = nc.declare_dram_parameter(
        "wt", [w_total], mybir.dt.uint8, isOutput=False)
    out = nc.declare_dram_parameter(
        "out", [MT * QT * 128, 512], mybir.dt.float16, isOutput=True)

    with TileContext(nc) as tc:
        with (
            tc.tile_pool(name="wq", bufs=1) as wq_pool,
            tc.tile_pool(name="xs", bufs=1) as xs_pool,
            tc.tile_pool(name="xpk", bufs=1) as xp_pool,
            tc.tile_pool(name="psum", bufs=8, space="PSUM") as psum_pool,
            tc.tile_pool(name="osb", bufs=8) as out_pool,
        ):
            xp_sb = xp_pool.tile([128, MT, 2, 128], mybir.dt.uint8)
            # kc-major so the pre-expanded xr chunks land per-partition
            # contiguous: xs_sb[:, kc, m0:m1] is one flat run.
            xs_sb = xs_pool.tile([128, KC, MT, 2, 128], mybir.dt.float8e4)
            wq_sb = wq_pool.tile([128, QT, KC, 2, 512], mybir.dt.float8e4)

            x_off = {}
            off = 0
            for b0, b1 in X_GROUPS:
                x_off[(b0, b1)] = off
                off += 128 * (b1 - b0) * 2 * 128
            w_off = {}
            off = 0
            for g in W_GROUPS:
                w_off[g] = off
                off += 128 * (g[2] - g[1]) * 2 * 512

            # Warmup scratch: memsets first on the DVE queue (its real work
            # waits on x DMA anyway), so scratch matmuls start ~1 us into
            # the window and keep the PE busy until real operands land --
            # an idle PE gap here would reset the HAM warm-up window.
            wu_a = wq_pool.tile([128, 2, 128], mybir.dt.float8e4)
            wu_b = wq_pool.tile([128, 2, 128], mybir.dt.float8e4)
            nc.vector.memset(wu_a, 0.0)
            nc.vector.memset(wu_b, 0.0)

            def dma_w(g):
                q, k0, k1 = g
                sz = 128 * (k1 - k0) * 2 * 512
                o0 = w_off[g]
                nc.sync.dma_start(
                    out=wq_sb[:, q, k0:k1].bitcast(mybir.dt.uint8),
                    in_=wt[o0:o0 + sz].rearrange("(p r) -> p r", p=128))

            def dma_x(b0, b1):
                sz = 128 * (b1 - b0) * 2 * 128
                o0 = x_off[(b0, b1)]
                nc.sync.dma_start(
                    out=xp_sb[:, b0:b1],
                    in_=xp[o0:o0 + sz].rearrange("(p r) -> p r", p=128))

            def dma_xr(kc):
                sz = 128 * MI_BLK * 2 * 128
                o0 = kc * sz
                nc.sync.dma_start(
                    out=xs_sb[:, kc, 0:MI_BLK].bitcast(mybir.dt.uint8),
                    in_=xr[o0:o0 + sz].rearrange("(p r) -> p r", p=128))

            # All loads trigger up front on the sync ring; it drains them
            # FIFO in this order, so the first-needed bytes own the wire and
            # the bulk quarters follow without flooding.
            for kind, g in LOAD_ORDER:
                if kind == "x":
                    dma_x(*g)
                elif kind == "r":
                    dma_xr(g)
                else:
                    dma_w(g)

            # PE warmup: scratch matmuls from ~+0.7 us keep the PE busy so
            # the HAM clock-gate fires (~+4.1 us) right as the real stream
            # gets going; sized to end as the first real operands land.
            wu_ps = psum_pool.tile([128, 128], mybir.dt.float32,
                                   name="wu_ps", tag="ps")
            for _ in range(WU_N):
                nc.tensor.matmul(wu_ps, wu_a, wu_b, start=True, stop=True,
                                 perf_mode=mybir.MatmulPerfMode.DoubleRow)

            xs_u32 = xs_sb.bitcast(mybir.dt.uint32)
            xp_u32 = xp_sb.bitcast(mybir.dt.uint32)

            def expand_x(m0, m1, kc):
                # Sign bits -> fp8 {+1,-1}: bit kc shifted to each byte's MSB
                # (fp8 sign bit), then OR in 0x38 (the fp8e4 encoding of 1.0).
                nc.vector.tensor_scalar(
                    out=xs_u32[:, kc, m0:m1], in0=xp_u32[:, m0:m1],
                    scalar1=7 - kc, scalar2=0x80808080,
                    op0=mybir.AluOpType.logical_shift_left,
                    op1=mybir.AluOpType.bitwise_and)
                nc.vector.tensor_scalar(
                    out=xs_u32[:, kc, m0:m1], in0=xs_u32[:, kc, m0:m1],
                    scalar1=0x38383838, scalar2=None,
                    op0=mybir.AluOpType.bitwise_or)

            for m0, m1, kc in E_HEAD:
                expand_x(m0, m1, kc)

            # PSUM drain: copy f32 -> f16 (exact) and store one contiguous
            # 128 KB block.  Copies alternate ACT/DVE and stores alternate
            # the sync/scalar rings so no single engine paces the drains.
            drain_ctr = [0]

            def drain(mi, q, ps, cols=None):
                blk = mi * QT + q
                k = drain_ctr[0]
                drain_ctr[0] += 1
                if cols is None:
                    ot = out_pool.tile([128, 512], mybir.dt.float16,
                                       name="ot", tag="ot")
                    dst = out[bass.ts(blk, 128)]
                else:
                    c0, c1 = cols
                    ot = out_pool.tile([128, c1 - c0], mybir.dt.float16,
                                       name="ot", tag="ot")
                    dst = out[bass.ts(blk, 128), c0:c1]
                if k % 2 == 0:
                    nc.scalar.copy(ot, ps)
                    nc.sync.dma_start(out=dst, in_=ot)
                else:
                    nc.vector.tensor_copy(ot, ps)
                    nc.scalar.dma_start(out=dst, in_=ot)

            def unit(mi, q):
                ps = psum_pool.tile([128, 512], mybir.dt.float32,
                                    name="ps", tag="ps")
                for kc in range(KC):
                    nc.tensor.matmul(
                        ps, xs_sb[:, kc, mi],
                        wq_sb[:, q, kc],
                        start=(kc == 0), stop=(kc == KC - 1),
                        perf_mode=mybir.MatmulPerfMode.DoubleRow)
                drain(mi, q, ps)

            def unit_split(mi, q):
                # Final unit: four independent column-slice accumulations so
                # the earlier slices' copy+store overlap the later slices'
                # matmuls, shrinking the end-of-kernel drain chain to one
                # 128-col copy + 32 KB store + receipt.
                for h in range(4):
                    ph = psum_pool.tile([128, 128], mybir.dt.float32,
                                        name="ps", tag="ps")
                    o0 = h * 128
                    for kc in range(KC):
                        nc.tensor.matmul(
                            ph, xs_sb[:, kc, mi],
                            wq_sb[:, q, kc, :, o0:o0 + 128],
                            start=(kc == 0), stop=(kc == KC - 1),
                            perf_mode=mybir.MatmulPerfMode.DoubleRow)
                    drain(mi, q, ph, cols=(o0, o0 + 128))

            # Block 0, quarter 0 runs kc-major: chunk kc sweeps mi0-7, so
            # matmuls start once (q0, kc0) and the first x group land; each
            # bank's last chunk is followed by its drain (staggered, one per
            # 8 matmuls).
            banks = [psum_pool.tile([128, 512], mybir.dt.float32,
                                    name=f"b{mi}", tag="ps")
                     for mi in range(MI_BLK)]
            for kc in range(KC):
                for mi in range(MI_BLK):
                    nc.tensor.matmul(
                        banks[mi], xs_sb[:, kc, mi], wq_sb[:, 0, kc],
                        start=(kc == 0), stop=(kc == KC - 1),
                        perf_mode=mybir.MatmulPerfMode.DoubleRow)
                    if kc == KC - 1:
                        drain(mi, 0, banks[mi])

            # Block 0, quarters 1-3 (weights long since resident): standard
            # per-unit accumulation.  The mi8-15 sign-bit expansion is
            # interleaved into the q2 phase (DVE is idle by then; the mi8-15
            # block starts ~+58 us).
            tail_iter = iter(E_TAIL)
            for q in range(1, QT):
                for mi in range(MI_BLK):
                    unit(mi, q)
                    if q == 2:
                        g = next(tail_iter, None)
                        if g is not None:
                            expand_x(*g)

            # Block 1: mi8-15, all quarters; last unit split for the tail.
            for q in range(QT):
                for mi in range(MI_BLK, MT):
                    if q == QT - 1 and mi == MT - 1:
                        unit_split(mi, q)
                    else:
                        unit(mi, q)

    nc.finalize()
    return nc


def ternarize_host(weight: np.ndarray) -> np.ndarray:
    """absmean ternarization, f64 for a faithful gamma; returns {-1,0,1} f32."""
    w = weight.astype(np.float64)
    gamma = np.mean(np.abs(w)) + EPS
    return (np.sign(w) * np.minimum(np.round(np.abs(w) / gamma), 1.0)).astype(
        np.float32)


def pack_w_flat(wq_t: np.ndarray) -> np.ndarray:
    """ternary Wq^T [i, o] f32 -> flat u8 (fp8e4 bytes), DMA-grouped."""
    # [kc, j, p, o] -> fp8 bytes
    w4 = wq_t.reshape(KC, 2, 128, O_DIM).astype(FP8).view(np.uint8)
    blocks = []
    for q, k0, k1 in W_GROUPS:
        blk = w4[k0:k1, :, :, q * 512:(q + 1) * 512]     # [kcr, 2, 128, 512]
        blocks.append(np.ascontiguousarray(
            blk.transpose(2, 0, 1, 3)).reshape(-1))      # partition-major
    return np.concatenate(blocks)


def pack_x_flat(sh: np.ndarray) -> np.ndarray:
    """x shard [m_per, i] f32 -> flat u8 sign-bit planes, DMA-grouped.

    Byte (p, mi, j, m) holds bit kc = signbit(x[mi*128+m, kc*256+j*128+p]).
    """
    sb = np.signbit(sh)                                   # [m, i] bool
    # [kc, j, p, mi, m] -> [p, mi, j, m, kc]
    b = sb.T.reshape(KC, 2, 128, MT, 128).transpose(2, 3, 1, 4, 0)
    pk = np.packbits(np.ascontiguousarray(b), axis=-1,
                     bitorder="little")[..., 0]           # [128, MT, 2, 128]
    blocks = [np.ascontiguousarray(pk[:, b0:b1]).reshape(-1)
              for b0, b1 in X_GROUPS]
    return np.concatenate(blocks)


def pack_xr_flat(sh: np.ndarray) -> np.ndarray:
    """Pre-expanded fp8 sign bytes for (kc < XR_KC, mi < MI_BLK).

    Byte (p, kc, mi, j, m) = 0x38 | 0x80*signbit(x[mi*128+m, kc*256+j*128+p])
    (fp8e4 +-1.0), flattened kc-group-major, partition-major per group.
    """
    sb = np.signbit(sh)                                   # [m, i] bool
    b5 = sb.T.reshape(KC, 2, 128, MT, 128)                # [kc, j, p, mi, m]
    blk = b5[:XR_KC, :, :, :MI_BLK]                       # [kcr, j, p, mi, m]
    byts = (0x38 | blk.astype(np.uint8) << 7)
    return np.ascontiguousarray(
        byts.transpose(0, 2, 3, 1, 4)).reshape(-1)        # [kcr, p, mi, j, m]


def prep_in_maps(x: np.ndarray, weight: np.ndarray) -> list[dict]:
    wq = ternarize_host(weight)                    # [o, i] ternary
    wt = pack_w_flat(np.ascontiguousarray(wq.T))
    xf = x.reshape(M_TOT, I_DIM)
    maps = []
    for c in range(N_CORES):
        sh = xf[c * M_PER:(c + 1) * M_PER]
        maps.append({"xp": pack_x_flat(sh), "xr": pack_xr_flat(sh),
                     "wt": wt})
    return maps


_PROGRAM_CACHE: dict = {}


def _get_program() -> bass.Bass:
    if "nc" not in _PROGRAM_CACHE:
        _PROGRAM_CACHE["nc"] = build_program()
    return _PROGRAM_CACHE["nc"]


def _gather(results: list[dict]) -> np.ndarray:
    # per-core out [MT*QT*128, 512] -> [m_per, o]
    shards = [
        np.asarray(r["out"]).reshape(MT, QT, 128, 512)
        .transpose(0, 2, 1, 3).reshape(M_PER, O_DIM)
        for r in results]
    full = np.concatenate(shards, axis=0)
    return np.ascontiguousarray(full.reshape(B, S, O_DIM).astype(np.float32))


def kernel(x: np.ndarray, weight: np.ndarray) -> np.ndarray:
    nc = _get_program()
    in_maps = prep_in_maps(np.asarray(x), np.asarray(weight))
    res = run_bass_kernel_spmd(nc, in_maps, core_ids=list(range(N_CORES)))
    return _gather(res.results)


def kernel_traced(x: np.ndarray, weight: np.ndarray, **trace_kw):
    """Like kernel() but returns (output, BassKernelResults) with a trace."""
    nc = _get_program()
    in_maps = prep_in_maps(np.asarray(x), np.asarray(weight))
    res = run_bass_kernel_spmd(
        nc, in_maps, core_ids=list(range(N_CORES)), trace=True, **trace_kw)
    return _gather(res.results), res
